# revision 1
# baseline (speedup 1.0000x reference)
"""Trainium2 Bass kernel for a 2-layer LSTM encoder/decoder forecaster.

Model (per batch element):
  teacher-forced over S=168 steps:  enc -> LSTM0 -> LSTM1 (keep last out)
  autoregressive rollout for 23 more steps feeding decoder output back.

Sharding: data-parallel, batch 1024 -> 8 cores x 128. 128 = partition width,
so each core's activations are single-partition-tile matrices. All weights
are replicated and stay resident in SBUF; zero inter-core communication.

Layout choice: gates are computed batch-major ([B=128, 4H]) with the
*activations* as the stationary matmul operand (lhsT, feature-major) and the
transposed weights as the moving operand (N=512 chunks). The encoder is
algebraically fused into layer 0 (M0 = Wih0 @ W_enc), its bias folded into an
appended ones-row of the transposed input, so a teacher-forced step is just
two gate matmul groups + the LSTM cell elementwise + one PE transpose per
layer. fp32r (TF32) is used for all matmuls: full PE rate at N>=512.
"""

import sys
import threading

sys.path.insert(0, "/opt/trn_rl_repo")

import numpy as np

PRED_LEN = 24
F, I, H = 64, 128, 512
B, S = 1024, 168
NCORES = 8
BL = B // NCORES          # batch per core = 128
G = 4 * H                 # gate width 2048
KX = F + 1                # x operand rows incl. ones row

_cache = {}
_cache_lock = threading.Lock()


def _gate_perm():
    # pytorch gate order i,f,g,o -> reorder rows to (i,f,o,g) so the three
    # sigmoid gates are contiguous for a single wide ACT op.
    return np.concatenate([
        np.arange(0, H),            # i
        np.arange(H, 2 * H),        # f
        np.arange(3 * H, 4 * H),    # o
        np.arange(2 * H, 3 * H),    # g
    ])


def _build_program(n_tf=S, n_ar=PRED_LEN - 1):
    import concourse.bacc as bacc
    import concourse.tile as tile
    import concourse.mybir as mybir

    F32 = mybir.dt.float32
    F32R = mybir.dt.float32r
    AF = mybir.ActivationFunctionType

    nc = bacc.Bacc("TRN2", target_bir_lowering=False, debug=False,
                   num_devices=NCORES)

    xT_d = nc.dram_tensor("xT", [n_tf, KX, BL], F32R, kind="ExternalInput").ap()
    m0_d = nc.dram_tensor("m0t", [KX, G], F32R, kind="ExternalInput").ap()
    whh0_d = nc.dram_tensor("whh0t", [H, G], F32R, kind="ExternalInput").ap()
    wih1_d = nc.dram_tensor("wih1t", [H, G], F32R, kind="ExternalInput").ap()
    whh1_d = nc.dram_tensor("whh1t", [H, G], F32R, kind="ExternalInput").ap()
    b1_d = nc.dram_tensor("b1", [1, G], F32R, kind="ExternalInput").ap()
    wdec_d = nc.dram_tensor("wdect", [H, F + 2], F32R, kind="ExternalInput").ap()
    bdec_d = nc.dram_tensor("bdec", [BL, F + 2], F32, kind="ExternalInput").ap()
    ones_d = nc.dram_tensor("ones", [1, BL], F32R, kind="ExternalInput").ap()
    ident_d = nc.dram_tensor("ident", [128, 128], F32R, kind="ExternalInput").ap()
    zeros_d = nc.dram_tensor("zeros", [128, H], F32R, kind="ExternalInput").ap()
    y_d = nc.dram_tensor("y", [n_ar + 1, BL, F], F32R, kind="ExternalOutput").ap()

    from contextlib import ExitStack
    with tile.TileContext(nc) as tc, ExitStack() as ctx:
        wpool = ctx.enter_context(tc.tile_pool(name="w", bufs=1))
        xpool = ctx.enter_context(tc.tile_pool(name="x", bufs=4))
        spool = ctx.enter_context(tc.tile_pool(name="s", bufs=2))
        hpool = ctx.enter_context(tc.tile_pool(name="h", bufs=2))
        pspool = ctx.enter_context(tc.tile_pool(name="ps", bufs=3, space="PSUM"))
        tppool = ctx.enter_context(tc.tile_pool(name="tp", bufs=2, space="PSUM"))

        # ---- resident weights ----
        m0_sb = wpool.tile([KX, G], F32R)
        nc.sync.dma_start(m0_sb[:], m0_d[:])
        # [H, G] weights stored k-chunk-major: [128, 4*G]
        whh0_sb = wpool.tile([128, 4 * G], F32R)
        wih1_sb = wpool.tile([128, 4 * G], F32R)
        whh1_sb = wpool.tile([128, 4 * G], F32R)
        for dst, srcd in ((whh0_sb, whh0_d), (wih1_sb, wih1_d), (whh1_sb, whh1_d)):
            for k in range(4):
                nc.sync.dma_start(dst[:, k * G:(k + 1) * G],
                                  srcd[k * 128:(k + 1) * 128, :])
        b1_sb = wpool.tile([1, G], F32R)
        nc.sync.dma_start(b1_sb[:], b1_d[:])
        wdec_sb = wpool.tile([128, 4 * (F + 2)], F32R)
        for k in range(4):
            nc.sync.dma_start(wdec_sb[:, k * (F + 2):(k + 1) * (F + 2)],
                              wdec_d[k * 128:(k + 1) * 128, :])
        bdec_sb = wpool.tile([BL, F + 2], F32)
        nc.sync.dma_start(bdec_sb[:], bdec_d[:])
        ones_sb = wpool.tile([1, BL], F32R)
        nc.sync.dma_start(ones_sb[:], ones_d[:])
        ident_sb = wpool.tile([128, 128], F32R)
        nc.sync.dma_start(ident_sb[:], ident_d[:])

        # ---- state ----
        h0T = hpool.tile([128, H], F32R, tag="h0T")
        nc.sync.dma_start(h0T[:], zeros_d[:])
        h1T = hpool.tile([128, H], F32R, tag="h1T")
        nc.sync.dma_start(h1T[:], zeros_d[:])
        c0 = hpool.tile([BL, H], F32, tag="c0")
        nc.gpsimd.memset(c0[:], 0.0)
        c1 = hpool.tile([BL, H], F32, tag="c1")
        nc.gpsimd.memset(c1[:], 0.0)

        tc.strict_bb_all_engine_barrier()

        def lstm_cell(gA, gB, c_prev, c_tag, h_tag, hT_tag):
            """gA = [i|f] psum, gB = [o|g] psum -> returns (c_new, hT_new)."""
            sig_if = spool.tile([BL, 2 * H], F32, tag="sif")
            nc.scalar.activation(sig_if[:], gA[:], AF.Sigmoid)
            sig_o = spool.tile([BL, H], F32, tag="so")
            nc.scalar.activation(sig_o[:], gB[:, 0:H], AF.Sigmoid)
            tanh_g = spool.tile([BL, H], F32, tag="tg")
            nc.scalar.activation(tanh_g[:], gB[:, H:2 * H], AF.Tanh)
            ig = spool.tile([BL, H], F32, tag="ig")
            nc.vector.tensor_mul(ig[:], sig_if[:, 0:H], tanh_g[:])
            fc = spool.tile([BL, H], F32, tag="fc")
            nc.vector.tensor_mul(fc[:], sig_if[:, H:2 * H], c_prev[:])
            c_new = hpool.tile([BL, H], F32, tag=c_tag)
            nc.vector.tensor_add(c_new[:], ig[:], fc[:])
            tanh_c = spool.tile([BL, H], F32, tag="tc")
            nc.scalar.activation(tanh_c[:], c_new[:], AF.Tanh)
            h_bm = spool.tile([BL, H], F32R, tag=h_tag)
            nc.vector.tensor_mul(h_bm[:], sig_o[:], tanh_c[:])
            # transpose h [B, H] -> hT [H-chunks, B]
            tp = tppool.tile([128, H], F32R, tag="tp")
            for k in range(4):
                nc.tensor.transpose(tp[:, k * 128:(k + 1) * 128],
                                    h_bm[:, k * 128:(k + 1) * 128], ident_sb[:])
            hT_new = hpool.tile([128, H], F32R, tag=hT_tag)
            nc.vector.tensor_copy(hT_new[:], tp[:])
            return c_new, hT_new

        n_steps = n_tf + n_ar
        xa_next = None  # AR input tile produced by previous step's decoder
        for t in range(n_steps):
            is_tf = t < n_tf
            if is_tf:
                xa = xpool.tile([KX, BL], F32R, tag="xa")
                nc.sync.dma_start(xa[:], xT_d[t])
            else:
                xa = xa_next

            # ---- layer 0 gates: chunks (0,1)=i,f -> gA0; (2,3)=o,g -> gB0
            gA0 = pspool.tile([BL, 2 * H], F32, tag="g")
            gB0 = pspool.tile([BL, 2 * H], F32, tag="g")
            for n in range(4):
                gt = gA0 if n < 2 else gB0
                psl = gt[:, (n % 2) * H:(n % 2 + 1) * H]
                wsl = slice(n * H, (n + 1) * H)
                nc.tensor.matmul(psl, xa[:], m0_sb[:, wsl], start=True, stop=False)
                for k in range(4):
                    nc.tensor.matmul(
                        psl, h0T[:, k * 128:(k + 1) * 128],
                        whh0_sb[:, k * G + n * H: k * G + (n + 1) * H],
                        start=False, stop=(k == 3))

            c0, h0T = lstm_cell(gA0, gB0, c0, "c0", "h0", "h0T")

            # ---- layer 1 gates: bias + h1 terms first (independent), h0 last
            gA1 = pspool.tile([BL, 2 * H], F32, tag="g")
            gB1 = pspool.tile([BL, 2 * H], F32, tag="g")
            for n in range(4):
                gt = gA1 if n < 2 else gB1
                psl = gt[:, (n % 2) * H:(n % 2 + 1) * H]
                wsl = slice(n * H, (n + 1) * H)
                nc.tensor.matmul(psl, ones_sb[:], b1_sb[:, wsl], start=True, stop=False)
                for k in range(4):
                    nc.tensor.matmul(
                        psl, h1T[:, k * 128:(k + 1) * 128],
                        whh1_sb[:, k * G + n * H: k * G + (n + 1) * H],
                        start=False, stop=False)
                for k in range(4):
                    nc.tensor.matmul(
                        psl, h0T[:, k * 128:(k + 1) * 128],
                        wih1_sb[:, k * G + n * H: k * G + (n + 1) * H],
                        start=False, stop=(k == 3))

            c1, h1T = lstm_cell(gA1, gB1, c1, "c1", "h1", "h1T")

            # ---- decoder (last TF step + all AR steps) ----
            if t >= n_tf - 1:
                j = t - (n_tf - 1)
                dps = tppool.tile([BL, F + 2], F32, tag="tp")
                for k in range(4):
                    nc.tensor.matmul(
                        dps[:], h1T[:, k * 128:(k + 1) * 128],
                        wdec_sb[:, k * (F + 2):(k + 1) * (F + 2)],
                        start=(k == 0), stop=(k == 3))
                dout = spool.tile([BL, F + 2], F32R, tag="dout")
                nc.vector.tensor_add(dout[:], dps[:], bdec_sb[:])
                nc.sync.dma_start(y_d[j], dout[:, 0:F])
                if j < n_ar:
                    tpx = tppool.tile([F + 2, BL], F32R, tag="tp")
                    nc.tensor.transpose(tpx[:], dout[:], ident_sb[:])
                    xa_next = xpool.tile([KX, BL], F32R, tag="xa")
                    nc.vector.tensor_copy(xa_next[:], tpx[0:KX, :])

    nc.compile()
    return nc


def _get_program(n_tf=S, n_ar=PRED_LEN - 1):
    key = (n_tf, n_ar)
    with _cache_lock:
        if key not in _cache:
            _cache[key] = _build_program(n_tf, n_ar)
        return _cache[key]


def _prep_weights(W_enc, b_enc, Wih0, Whh0, bih0, bhh0,
                  Wih1, Whh1, bih1, bhh1, W_dec, b_dec):
    perm = _gate_perm()
    f32 = np.float32

    M0 = (Wih0 @ W_enc)[perm]                                   # [G, F]
    b0 = (Wih0 @ b_enc + bih0 + bhh0)[perm]                     # [G]
    m0t = np.concatenate([M0.T, b0[None, :]], axis=0)           # [KX, G]

    whh0t = np.ascontiguousarray(Whh0[perm].T)                  # [H, G]
    wih1t = np.ascontiguousarray(Wih1[perm].T)                  # [H, G]
    whh1t = np.ascontiguousarray(Whh1[perm].T)                  # [H, G]
    b1 = (bih1 + bhh1)[perm][None, :]                           # [1, G]

    wdect = np.concatenate([W_dec.T, np.zeros((H, 2), f32)], axis=1)  # [H, F+2]
    bdec = np.concatenate([b_dec, np.ones((1,), f32), np.zeros((1,), f32)])
    bdec_b = np.broadcast_to(bdec[None, :], (BL, F + 2)).copy() # [BL, F+2]

    return {
        "m0t": np.ascontiguousarray(m0t, f32),
        "whh0t": whh0t.astype(f32),
        "wih1t": wih1t.astype(f32),
        "whh1t": whh1t.astype(f32),
        "b1": np.ascontiguousarray(b1, f32),
        "wdect": np.ascontiguousarray(wdect, f32),
        "bdec": bdec_b.astype(f32),
        "ones": np.ones((1, BL), f32),
        "ident": np.eye(128, dtype=f32),
        "zeros": np.zeros((128, H), f32),
    }


def kernel(x, W_enc, b_enc, Wih0, Whh0, bih0, bhh0,
           Wih1, Whh1, bih1, bhh1, W_dec, b_dec, _n_tf=S, _n_ar=PRED_LEN - 1):
    from concourse.bass_utils import run_bass_kernel_spmd

    x = np.asarray(x, np.float32)
    weights = _prep_weights(
        np.asarray(W_enc, np.float32), np.asarray(b_enc, np.float32),
        np.asarray(Wih0, np.float32), np.asarray(Whh0, np.float32),
        np.asarray(bih0, np.float32), np.asarray(bhh0, np.float32),
        np.asarray(Wih1, np.float32), np.asarray(Whh1, np.float32),
        np.asarray(bih1, np.float32), np.asarray(bhh1, np.float32),
        np.asarray(W_dec, np.float32), np.asarray(b_dec, np.float32))

    nc = _get_program(_n_tf, _n_ar)

    in_maps = []
    for c in range(NCORES):
        xs = x[c * BL:(c + 1) * BL, :_n_tf, :]                # [BL, n_tf, F]
        xT = np.ascontiguousarray(xs.transpose(1, 2, 0))      # [n_tf, F, BL]
        xa = np.concatenate(
            [xT, np.ones((_n_tf, 1, BL), np.float32)], axis=1)  # [n_tf, KX, BL]
        in_maps.append({"xT": np.ascontiguousarray(xa), **weights})

    res = run_bass_kernel_spmd(nc, in_maps, core_ids=list(range(NCORES)))

    out = np.empty((B, _n_ar + 1, F), np.float32)
    for c in range(NCORES):
        y = res.results[c]["y"]                               # [n_ar+1, BL, F]
        out[c * BL:(c + 1) * BL] = y.transpose(1, 0, 2)
    return out



# revision 2
# speedup vs baseline: 849.3906x; 849.3906x over previous
"""Trainium2 Bass kernel for a 2-layer LSTM encoder/decoder forecaster.

Model (per batch element):
  teacher-forced over S=168 steps:  enc -> LSTM0 -> LSTM1 (keep last out)
  autoregressive rollout for 23 more steps feeding decoder output back.

Sharding: data-parallel, batch 1024 -> 8 cores x 128. 128 = partition width,
so each core's activations are single-partition-tile matrices. All weights
are replicated and stay resident in SBUF; zero inter-core communication.

Layout choice: gates are computed batch-major ([B=128, 4H]) with the
*activations* as the stationary matmul operand (lhsT, feature-major) and the
transposed weights as the moving operand (N=512 chunks). The encoder is
algebraically fused into layer 0 (M0 = Wih0 @ W_enc), its bias folded into an
appended ones-row of the transposed input, so a teacher-forced step is just
two gate matmul groups + the LSTM cell elementwise + one PE transpose per
layer. fp32r (TF32) is used for all matmuls: full PE rate at N>=512.
"""

import sys
import threading

sys.path.insert(0, "/opt/trn_rl_repo")

import numpy as np

PRED_LEN = 24
F, I, H = 64, 128, 512
B, S = 1024, 168
NCORES = 8
BL = B // NCORES          # batch per core = 128
G = 4 * H                 # gate width 2048
KX = F + 1                # x operand rows incl. ones row

_cache = {}
_cache_lock = threading.Lock()


def _gate_perm():
    # pytorch gate order i,f,g,o -> reorder rows to (i,f,o,g) so the three
    # sigmoid gates are contiguous for a single wide ACT op.
    return np.concatenate([
        np.arange(0, H),            # i
        np.arange(H, 2 * H),        # f
        np.arange(3 * H, 4 * H),    # o
        np.arange(2 * H, 3 * H),    # g
    ])


def _build_program(n_tf=S, n_ar=PRED_LEN - 1):
    import concourse.bacc as bacc
    import concourse.tile as tile
    import concourse.mybir as mybir

    F32 = mybir.dt.float32
    F32R = mybir.dt.float32r
    AF = mybir.ActivationFunctionType

    nc = bacc.Bacc("TRN2", target_bir_lowering=False, debug=False,
                   num_devices=NCORES)

    xT_d = nc.dram_tensor("xT", [n_tf, KX, BL], F32R, kind="ExternalInput").ap()
    m0_d = nc.dram_tensor("m0t", [KX, G], F32R, kind="ExternalInput").ap()
    whh0_d = nc.dram_tensor("whh0t", [H, G], F32R, kind="ExternalInput").ap()
    wih1_d = nc.dram_tensor("wih1t", [H, G], F32R, kind="ExternalInput").ap()
    whh1_d = nc.dram_tensor("whh1t", [H, G], F32R, kind="ExternalInput").ap()
    b1_d = nc.dram_tensor("b1", [1, G], F32R, kind="ExternalInput").ap()
    wdec_d = nc.dram_tensor("wdect", [H, F + 2], F32R, kind="ExternalInput").ap()
    bdec_d = nc.dram_tensor("bdec", [BL, F + 2], F32, kind="ExternalInput").ap()
    ones_d = nc.dram_tensor("ones", [1, BL], F32R, kind="ExternalInput").ap()
    ident_d = nc.dram_tensor("ident", [128, 128], F32R, kind="ExternalInput").ap()
    zeros_d = nc.dram_tensor("zeros", [128, H], F32R, kind="ExternalInput").ap()
    y_d = nc.dram_tensor("y", [n_ar + 1, BL, F], F32R, kind="ExternalOutput").ap()

    from contextlib import ExitStack
    with tile.TileContext(nc) as tc, ExitStack() as ctx:
        wpool = ctx.enter_context(tc.tile_pool(name="w", bufs=1))
        xpool = ctx.enter_context(tc.tile_pool(name="x", bufs=4))
        spool = ctx.enter_context(tc.tile_pool(name="s", bufs=2))
        hpool = ctx.enter_context(tc.tile_pool(name="h", bufs=2))
        pspool = ctx.enter_context(tc.tile_pool(name="ps", bufs=3, space="PSUM"))
        tppool = ctx.enter_context(tc.tile_pool(name="tp", bufs=2, space="PSUM"))

        # ---- resident weights ----
        m0_sb = wpool.tile([KX, G], F32R)
        nc.sync.dma_start(m0_sb[:], m0_d[:])
        # [H, G] weights stored k-chunk-major: [128, 4*G]
        whh0_sb = wpool.tile([128, 4 * G], F32R)
        wih1_sb = wpool.tile([128, 4 * G], F32R)
        whh1_sb = wpool.tile([128, 4 * G], F32R)
        for dst, srcd in ((whh0_sb, whh0_d), (wih1_sb, wih1_d), (whh1_sb, whh1_d)):
            for k in range(4):
                nc.sync.dma_start(dst[:, k * G:(k + 1) * G],
                                  srcd[k * 128:(k + 1) * 128, :])
        b1_sb = wpool.tile([1, G], F32R)
        nc.sync.dma_start(b1_sb[:], b1_d[:])
        wdec_sb = wpool.tile([128, 4 * (F + 2)], F32R)
        for k in range(4):
            nc.sync.dma_start(wdec_sb[:, k * (F + 2):(k + 1) * (F + 2)],
                              wdec_d[k * 128:(k + 1) * 128, :])
        bdec_sb = wpool.tile([BL, F + 2], F32)
        nc.sync.dma_start(bdec_sb[:], bdec_d[:])
        ones_sb = wpool.tile([1, BL], F32R)
        nc.sync.dma_start(ones_sb[:], ones_d[:])
        ident_sb = wpool.tile([128, 128], F32R)
        nc.sync.dma_start(ident_sb[:], ident_d[:])

        # ---- state ----
        h0T = hpool.tile([128, H], F32R, tag="h0T")
        nc.sync.dma_start(h0T[:], zeros_d[:])
        h1T = hpool.tile([128, H], F32R, tag="h1T")
        nc.sync.dma_start(h1T[:], zeros_d[:])
        c0 = hpool.tile([BL, H], F32, tag="c0")
        nc.gpsimd.memset(c0[:], 0.0)
        c1 = hpool.tile([BL, H], F32, tag="c1")
        nc.gpsimd.memset(c1[:], 0.0)

        tc.strict_bb_all_engine_barrier()

        def lstm_cell(gA, gB, c_prev, c_tag, h_tag, hT_tag):
            """gA = [i|f] psum, gB = [o|g] psum -> returns (c_new, hT_new)."""
            sig_if = spool.tile([BL, 2 * H], F32, tag="sif")
            nc.scalar.activation(sig_if[:], gA[:], AF.Sigmoid)
            sig_o = spool.tile([BL, H], F32, tag="so")
            nc.scalar.activation(sig_o[:], gB[:, 0:H], AF.Sigmoid)
            tanh_g = spool.tile([BL, H], F32, tag="tg")
            nc.scalar.activation(tanh_g[:], gB[:, H:2 * H], AF.Tanh)
            ig = spool.tile([BL, H], F32, tag="ig")
            nc.vector.tensor_mul(ig[:], sig_if[:, 0:H], tanh_g[:])
            fc = spool.tile([BL, H], F32, tag="fc")
            nc.vector.tensor_mul(fc[:], sig_if[:, H:2 * H], c_prev[:])
            c_new = hpool.tile([BL, H], F32, tag=c_tag)
            nc.vector.tensor_add(c_new[:], ig[:], fc[:])
            tanh_c = spool.tile([BL, H], F32, tag="tc")
            nc.scalar.activation(tanh_c[:], c_new[:], AF.Tanh)
            h_bm = spool.tile([BL, H], F32R, tag=h_tag)
            nc.vector.tensor_mul(h_bm[:], sig_o[:], tanh_c[:])
            # transpose h [B, H] -> hT [H-chunks, B]
            tp = tppool.tile([128, H], F32R, tag="tp")
            for k in range(4):
                nc.tensor.transpose(tp[:, k * 128:(k + 1) * 128],
                                    h_bm[:, k * 128:(k + 1) * 128], ident_sb[:])
            hT_new = hpool.tile([128, H], F32R, tag=hT_tag)
            nc.vector.tensor_copy(hT_new[:], tp[:])
            return c_new, hT_new

        n_steps = n_tf + n_ar
        xa_next = None  # AR input tile produced by previous step's decoder
        for t in range(n_steps):
            is_tf = t < n_tf
            if is_tf:
                xa = xpool.tile([KX, BL], F32R, tag="xa")
                nc.sync.dma_start(xa[:], xT_d[t])
            else:
                xa = xa_next

            # ---- layer 0 gates: chunks (0,1)=i,f -> gA0; (2,3)=o,g -> gB0
            gA0 = pspool.tile([BL, 2 * H], F32, tag="g")
            gB0 = pspool.tile([BL, 2 * H], F32, tag="g")
            for n in range(4):
                gt = gA0 if n < 2 else gB0
                psl = gt[:, (n % 2) * H:(n % 2 + 1) * H]
                wsl = slice(n * H, (n + 1) * H)
                nc.tensor.matmul(psl, xa[:], m0_sb[:, wsl], start=True, stop=False)
                for k in range(4):
                    nc.tensor.matmul(
                        psl, h0T[:, k * 128:(k + 1) * 128],
                        whh0_sb[:, k * G + n * H: k * G + (n + 1) * H],
                        start=False, stop=(k == 3))

            c0, h0T = lstm_cell(gA0, gB0, c0, "c0", "h0", "h0T")

            # ---- layer 1 gates: bias + h1 terms first (independent), h0 last
            gA1 = pspool.tile([BL, 2 * H], F32, tag="g")
            gB1 = pspool.tile([BL, 2 * H], F32, tag="g")
            for n in range(4):
                gt = gA1 if n < 2 else gB1
                psl = gt[:, (n % 2) * H:(n % 2 + 1) * H]
                wsl = slice(n * H, (n + 1) * H)
                nc.tensor.matmul(psl, ones_sb[:], b1_sb[:, wsl], start=True, stop=False)
                for k in range(4):
                    nc.tensor.matmul(
                        psl, h1T[:, k * 128:(k + 1) * 128],
                        whh1_sb[:, k * G + n * H: k * G + (n + 1) * H],
                        start=False, stop=False)
                for k in range(4):
                    nc.tensor.matmul(
                        psl, h0T[:, k * 128:(k + 1) * 128],
                        wih1_sb[:, k * G + n * H: k * G + (n + 1) * H],
                        start=False, stop=(k == 3))

            c1, h1T = lstm_cell(gA1, gB1, c1, "c1", "h1", "h1T")

            # ---- decoder (last TF step + all AR steps) ----
            if t >= n_tf - 1:
                j = t - (n_tf - 1)
                dps = tppool.tile([BL, F + 2], F32, tag="tp")
                for k in range(4):
                    nc.tensor.matmul(
                        dps[:], h1T[:, k * 128:(k + 1) * 128],
                        wdec_sb[:, k * (F + 2):(k + 1) * (F + 2)],
                        start=(k == 0), stop=(k == 3))
                dout = spool.tile([BL, F + 2], F32R, tag="dout")
                nc.vector.tensor_add(dout[:], dps[:], bdec_sb[:])
                nc.sync.dma_start(y_d[j], dout[:, 0:F])
                if j < n_ar:
                    tpx = tppool.tile([F + 2, BL], F32R, tag="tp")
                    nc.tensor.transpose(tpx[:], dout[:], ident_sb[:])
                    xa_next = xpool.tile([KX, BL], F32R, tag="xa")
                    nc.vector.tensor_copy(xa_next[:], tpx[0:KX, :])

    nc.compile()
    return nc


def _get_program(n_tf=S, n_ar=PRED_LEN - 1):
    key = (n_tf, n_ar)
    with _cache_lock:
        if key not in _cache:
            _cache[key] = _build_program(n_tf, n_ar)
        return _cache[key]


def _prep_weights(W_enc, b_enc, Wih0, Whh0, bih0, bhh0,
                  Wih1, Whh1, bih1, bhh1, W_dec, b_dec):
    perm = _gate_perm()
    f32 = np.float32

    M0 = (Wih0 @ W_enc)[perm]                                   # [G, F]
    b0 = (Wih0 @ b_enc + bih0 + bhh0)[perm]                     # [G]
    m0t = np.concatenate([M0.T, b0[None, :]], axis=0)           # [KX, G]

    whh0t = np.ascontiguousarray(Whh0[perm].T)                  # [H, G]
    wih1t = np.ascontiguousarray(Wih1[perm].T)                  # [H, G]
    whh1t = np.ascontiguousarray(Whh1[perm].T)                  # [H, G]
    b1 = (bih1 + bhh1)[perm][None, :]                           # [1, G]

    wdect = np.concatenate([W_dec.T, np.zeros((H, 2), f32)], axis=1)  # [H, F+2]
    bdec = np.concatenate([b_dec, np.ones((1,), f32), np.zeros((1,), f32)])
    bdec_b = np.broadcast_to(bdec[None, :], (BL, F + 2)).copy() # [BL, F+2]

    return {
        "m0t": np.ascontiguousarray(m0t, f32),
        "whh0t": whh0t.astype(f32),
        "wih1t": wih1t.astype(f32),
        "whh1t": whh1t.astype(f32),
        "b1": np.ascontiguousarray(b1, f32),
        "wdect": np.ascontiguousarray(wdect, f32),
        "bdec": bdec_b.astype(f32),
        "ones": np.ones((1, BL), f32),
        "ident": np.eye(128, dtype=f32),
        "zeros": np.zeros((128, H), f32),
    }


def _make_in_maps(x, weights, _n_tf=S):
    in_maps = []
    for c in range(NCORES):
        xs = x[c * BL:(c + 1) * BL, :_n_tf, :]                # [BL, n_tf, F]
        xT = np.ascontiguousarray(xs.transpose(1, 2, 0))      # [n_tf, F, BL]
        xa = np.concatenate(
            [xT, np.ones((_n_tf, 1, BL), np.float32)], axis=1)  # [n_tf, KX, BL]
        in_maps.append({"xT": np.ascontiguousarray(xa), **weights})
    return in_maps


def kernel(x, W_enc, b_enc, Wih0, Whh0, bih0, bhh0,
           Wih1, Whh1, bih1, bhh1, W_dec, b_dec, _n_tf=S, _n_ar=PRED_LEN - 1):
    from concourse.bass_utils import run_bass_kernel_spmd

    x = np.asarray(x, np.float32)
    weights = _prep_weights(
        np.asarray(W_enc, np.float32), np.asarray(b_enc, np.float32),
        np.asarray(Wih0, np.float32), np.asarray(Whh0, np.float32),
        np.asarray(bih0, np.float32), np.asarray(bhh0, np.float32),
        np.asarray(Wih1, np.float32), np.asarray(Whh1, np.float32),
        np.asarray(bih1, np.float32), np.asarray(bhh1, np.float32),
        np.asarray(W_dec, np.float32), np.asarray(b_dec, np.float32))

    nc = _get_program(_n_tf, _n_ar)
    in_maps = _make_in_maps(x, weights, _n_tf)
    res = run_bass_kernel_spmd(nc, in_maps, core_ids=list(range(NCORES)))

    out = np.empty((B, _n_ar + 1, F), np.float32)
    for c in range(NCORES):
        y = res.results[c]["y"]                               # [n_ar+1, BL, F]
        out[c * BL:(c + 1) * BL] = y.transpose(1, 0, 2)
    return out



# revision 4
# speedup vs baseline: 1230.5870x; 1.4488x over previous
"""Trainium2 Bass kernel for a 2-layer LSTM encoder/decoder forecaster.

Model (per batch element):
  teacher-forced over S=168 steps:  enc -> LSTM0 -> LSTM1 (keep last out)
  autoregressive rollout for 23 more steps feeding decoder output back.

Sharding: data-parallel, batch 1024 -> 8 cores x 128. All weights are
replicated and SBUF-resident; zero inter-core communication.

Layout: everything is FEATURE-MAJOR. Gates are computed as 16 chunks of
[128 gate-rows x 128 batch] PSUM tiles, with the (transposed, chunked)
weights as the stationary matmul operand and h / x as the moving
operand. Because the cell elementwise output h = sig(o)*tanh(c) is then
produced directly in [feature, batch] layout, it is ALREADY the k-chunk
operand the next step's recurrent matmuls need - no PE transposes, no
PSUM->SBUF copies of state anywhere in the loop.

Everything is bf16 on the matmul path (full PE rate at any width, and a
validated ~3e-3 end-to-end error vs the 2e-2 budget; fp8 was measured at
4.3e-2 and rejected). The cell state c stays fp32.

Gate chunk order after host-side row permutation: (f, i, o, g) so that
GA = [f|i] (chunks 0..7) and GB = [o|g] (chunks 8..15), letting each
activation read one contiguous PSUM span.

The encoder is algebraically fused into layer 0 (M0 = Wih0 @ W_enc, bias
folded into an appended ones-row of the feature-major input). Layer 1's
bias enters via 16 K=1 matmuls against a resident ones vector. The
decoder is augmented with a column that regenerates the ones-row so the
AR feedback tile needs no fixup at all: the decoder's SBUF output IS the
next step's input operand.

PE order per steady-state step: [xa(t), whh0(t), bias1(t)] prefetched at
the end of step t-1, then whh1(t), wih1(t) - chosen so every matmul's
input is ready before PE reaches it, keeping PE (the bottleneck at ~28.7k
cycles/step) gapless and at full p-state.
"""

import sys
import threading

sys.path.insert(0, "/opt/trn_rl_repo")

import numpy as np
import ml_dtypes

PRED_LEN = 24
F, I, H = 64, 128, 512
B, S = 1024, 168
NCORES = 8
BL = B // NCORES          # batch per core = 128
G = 4 * H                 # gate width 2048
NCH = G // 128            # 16 gate chunks
KCH = H // 128            # 4 k-chunks of the hidden dim
KX = F + 1                # x operand rows incl. ones row = 65
FD = F + 2                # decoder rows: 64 outputs + ones + pad = 66

BF16NP = ml_dtypes.bfloat16

_cache = {}
_cache_lock = threading.Lock()


def _gate_perm():
    # pytorch gate order i,f,g,o -> reorder rows to (f,i,o,g): chunks
    # 0-3=f, 4-7=i (-> GA), 8-11=o, 12-15=g (-> GB).
    return np.concatenate([
        np.arange(H, 2 * H),        # f
        np.arange(0, H),            # i
        np.arange(3 * H, 4 * H),    # o
        np.arange(2 * H, 3 * H),    # g
    ])


def _build_program(n_tf=S, n_ar=PRED_LEN - 1):
    import concourse.bacc as bacc
    import concourse.tile as tile
    import concourse.mybir as mybir

    F32 = mybir.dt.float32
    BF16 = mybir.dt.bfloat16
    AF = mybir.ActivationFunctionType

    nc = bacc.Bacc("TRN2", target_bir_lowering=False, debug=False,
                   num_devices=NCORES)

    x_d = nc.dram_tensor("xT", [KX, n_tf, BL], BF16, kind="ExternalInput").ap()
    m0_d = nc.dram_tensor("m0t", [KX, G], BF16, kind="ExternalInput").ap()
    whh0_d = nc.dram_tensor("whh0t", [128, KCH, G], BF16, kind="ExternalInput").ap()
    wih1_d = nc.dram_tensor("wih1t", [128, KCH, G], BF16, kind="ExternalInput").ap()
    whh1_d = nc.dram_tensor("whh1t", [128, KCH, G], BF16, kind="ExternalInput").ap()
    b1_d = nc.dram_tensor("b1", [1, G], BF16, kind="ExternalInput").ap()
    ones_d = nc.dram_tensor("ones", [1, BL], BF16, kind="ExternalInput").ap()
    wdec_d = nc.dram_tensor("wdect", [128, KCH, FD], BF16, kind="ExternalInput").ap()
    bdec_d = nc.dram_tensor("bdec", [FD, 1], F32, kind="ExternalInput").ap()
    y_d = nc.dram_tensor("y", [n_ar + 1, F, BL], BF16, kind="ExternalOutput").ap()

    from contextlib import ExitStack
    with tile.TileContext(nc) as tc, ExitStack() as ctx:
        wpool = ctx.enter_context(tc.tile_pool(name="w", bufs=1))
        spool = ctx.enter_context(tc.tile_pool(name="s", bufs=2))
        hpool = ctx.enter_context(tc.tile_pool(name="h", bufs=2))
        dpool = ctx.enter_context(tc.tile_pool(name="d", bufs=2))
        gpool = ctx.enter_context(tc.tile_pool(name="g", bufs=1, space="PSUM"))

        # ---- resident weights + input ----
        x_sb = wpool.tile([KX, n_tf, BL], BF16)
        nc.sync.dma_start(x_sb[:], x_d[:])
        m0_sb = wpool.tile([KX, G], BF16)
        nc.sync.dma_start(m0_sb[:], m0_d[:])
        whh0_sb = wpool.tile([128, KCH, G], BF16)
        nc.sync.dma_start(whh0_sb[:], whh0_d[:])
        wih1_sb = wpool.tile([128, KCH, G], BF16)
        nc.sync.dma_start(wih1_sb[:], wih1_d[:])
        whh1_sb = wpool.tile([128, KCH, G], BF16)
        nc.sync.dma_start(whh1_sb[:], whh1_d[:])
        b1_sb = wpool.tile([1, G], BF16)
        nc.sync.dma_start(b1_sb[:], b1_d[:])
        ones_sb = wpool.tile([1, BL], BF16)
        nc.sync.dma_start(ones_sb[:], ones_d[:])
        wdec_sb = wpool.tile([128, KCH, FD], BF16)
        nc.sync.dma_start(wdec_sb[:], wdec_d[:])
        bdec_sb = wpool.tile([FD, 1], F32)
        nc.sync.dma_start(bdec_sb[:], bdec_d[:])

        def halves(ga, gb, m):
            return (ga if m < 8 else gb)[:, m % 8, :]

        # start=True lazily zeroes the WHOLE 2KB PSUM bank (4 of our 512B
        # chunk regions), so only the first write per bank may set it; the
        # sibling regions still see the bank's pending-zero and overwrite.
        def emit_xa(ga, gb, rhs, only):
            # input-side gate contribution; `only`=True closes the group
            # (t=0 has no recurrent term).
            for m in range(NCH):
                nc.tensor.matmul(halves(ga, gb, m),
                                 m0_sb[:, m * 128:(m + 1) * 128], rhs,
                                 start=(m % 4 == 0), stop=only,
                                 skip_group_check=True)

        def emit_bias1(ga, gb, only):
            for m in range(NCH):
                nc.tensor.matmul(halves(ga, gb, m),
                                 b1_sb[:, m * 128:(m + 1) * 128], ones_sb[:],
                                 start=(m % 4 == 0), stop=only,
                                 skip_group_check=True)

        def emit_rec(w_sb, h, ga, gb, last):
            for m in range(NCH):
                out = halves(ga, gb, m)
                for k in range(KCH):
                    nc.tensor.matmul(out,
                                     w_sb[:, k, m * 128:(m + 1) * 128],
                                     h[:, k, :],
                                     start=False, stop=(last and k == KCH - 1),
                                     skip_group_check=True)

        def cell(ga, gb, c_prev, l):
            sig_fi = spool.tile([128, 8, BL], F32, tag=f"sfi{l}")
            nc.scalar.activation(sig_fi[:], ga[:], AF.Sigmoid)
            tanh_g = spool.tile([128, KCH, BL], F32, tag=f"tg{l}")
            nc.scalar.activation(tanh_g[:], gb[:, 4:8, :], AF.Tanh)
            sig_o = spool.tile([128, KCH, BL], F32, tag=f"so{l}")
            nc.scalar.activation(sig_o[:], gb[:, 0:4, :], AF.Sigmoid)
            ig = spool.tile([128, KCH, BL], F32, tag=f"ig{l}")
            nc.vector.tensor_mul(ig[:], sig_fi[:, 4:8, :], tanh_g[:])
            c_new = hpool.tile([128, KCH, BL], F32, tag=f"c{l}")
            if c_prev is None:
                nc.vector.tensor_copy(c_new[:], ig[:])
            else:
                fc = spool.tile([128, KCH, BL], F32, tag=f"fc{l}")
                nc.vector.tensor_mul(fc[:], sig_fi[:, 0:4, :], c_prev[:])
                nc.vector.tensor_add(c_new[:], fc[:], ig[:])
            tanh_c = spool.tile([128, KCH, BL], F32, tag=f"tc{l}")
            nc.scalar.activation(tanh_c[:], c_new[:], AF.Tanh)
            h_new = hpool.tile([128, KCH, BL], BF16, tag=f"h{l}")
            nc.vector.tensor_mul(h_new[:], sig_o[:], tanh_c[:])
            return c_new, h_new

        n_steps = n_tf + n_ar
        h0 = h1 = c0 = c1 = None
        dout = None
        ga0 = gb0 = ga1 = gb1 = None
        for t in range(n_steps):
            if t == 0:
                ga0 = gpool.tile([128, 8, BL], F32, tag="ga0")
                gb0 = gpool.tile([128, 8, BL], F32, tag="gb0")
                emit_xa(ga0, gb0, x_sb[:, 0, :], only=True)
            c0, h0 = cell(ga0, gb0, c0, 0)
            if t == 0:
                ga1 = gpool.tile([128, 8, BL], F32, tag="ga1")
                gb1 = gpool.tile([128, 8, BL], F32, tag="gb1")
                emit_bias1(ga1, gb1, only=False)
            else:
                emit_rec(whh1_sb, h1, ga1, gb1, last=False)
            emit_rec(wih1_sb, h0, ga1, gb1, last=True)
            c1, h1 = cell(ga1, gb1, c1, 1)

            if t >= n_tf - 1:
                j = t - (n_tf - 1)
                dec_ps = gpool.tile([FD, BL], F32, tag="gb1")
                for k in range(KCH):
                    nc.tensor.matmul(dec_ps[:], wdec_sb[:, k, :], h1[:, k, :],
                                     start=(k == 0), stop=(k == KCH - 1))
                dout = dpool.tile([FD, BL], BF16, tag="dout")
                nc.scalar.add(dout[:], dec_ps[:], bdec_sb[:])
                nc.sync.dma_start(y_d[j], dout[0:F, :])

            if t + 1 < n_steps:
                ga0 = gpool.tile([128, 8, BL], F32, tag="ga0")
                gb0 = gpool.tile([128, 8, BL], F32, tag="gb0")
                rhs = x_sb[:, t + 1, :] if t + 1 < n_tf else dout[0:KX, :]
                emit_xa(ga0, gb0, rhs, only=False)
                emit_rec(whh0_sb, h0, ga0, gb0, last=True)
                ga1 = gpool.tile([128, 8, BL], F32, tag="ga1")
                gb1 = gpool.tile([128, 8, BL], F32, tag="gb1")
                emit_bias1(ga1, gb1, only=False)

    nc.compile()
    return nc


def _get_program(n_tf=S, n_ar=PRED_LEN - 1):
    key = (n_tf, n_ar)
    with _cache_lock:
        if key not in _cache:
            _cache[key] = _build_program(n_tf, n_ar)
        return _cache[key]


def _kmajor(w):
    """[H, N] -> [128, KCH, N]: row h = k*128 + p lands at [p, k, :]."""
    n = w.shape[1]
    return np.ascontiguousarray(
        w.reshape(KCH, 128, n).transpose(1, 0, 2)).astype(BF16NP)


def _prep_weights(W_enc, b_enc, Wih0, Whh0, bih0, bhh0,
                  Wih1, Whh1, bih1, bhh1, W_dec, b_dec):
    perm = _gate_perm()
    f32 = np.float32

    M0 = (Wih0 @ W_enc)[perm]                                   # [G, F]
    b0 = (Wih0 @ b_enc + bih0 + bhh0)[perm]                     # [G]
    m0t = np.concatenate([M0.T, b0[None, :]], axis=0)           # [KX, G]

    wdec_aug = np.concatenate(
        [W_dec.T, np.zeros((H, 2), f32)], axis=1)               # [H, FD]
    bdec = np.concatenate([b_dec, np.ones((1,), f32), np.zeros((1,), f32)])

    return {
        "m0t": np.ascontiguousarray(m0t).astype(BF16NP),
        "whh0t": _kmajor(np.ascontiguousarray(Whh0[perm].T)),
        "wih1t": _kmajor(np.ascontiguousarray(Wih1[perm].T)),
        "whh1t": _kmajor(np.ascontiguousarray(Whh1[perm].T)),
        "b1": ((bih1 + bhh1)[perm][None, :]).astype(BF16NP),
        "ones": np.ones((1, BL), BF16NP),
        "wdect": _kmajor(wdec_aug),
        "bdec": np.ascontiguousarray(bdec[:, None], f32),
    }


def _make_in_maps(x, weights, _n_tf=S):
    in_maps = []
    for c in range(NCORES):
        xs = x[c * BL:(c + 1) * BL, :_n_tf, :]                # [BL, n_tf, F]
        xT = xs.transpose(2, 1, 0)                            # [F, n_tf, BL]
        xa = np.concatenate(
            [xT, np.ones((1, _n_tf, BL), np.float32)], axis=0)  # [KX, n_tf, BL]
        in_maps.append(
            {"xT": np.ascontiguousarray(xa).astype(BF16NP), **weights})
    return in_maps


def kernel(x, W_enc, b_enc, Wih0, Whh0, bih0, bhh0,
           Wih1, Whh1, bih1, bhh1, W_dec, b_dec, _n_tf=S, _n_ar=PRED_LEN - 1):
    from concourse.bass_utils import run_bass_kernel_spmd

    x = np.asarray(x, np.float32)
    weights = _prep_weights(
        np.asarray(W_enc, np.float32), np.asarray(b_enc, np.float32),
        np.asarray(Wih0, np.float32), np.asarray(Whh0, np.float32),
        np.asarray(bih0, np.float32), np.asarray(bhh0, np.float32),
        np.asarray(Wih1, np.float32), np.asarray(Whh1, np.float32),
        np.asarray(bih1, np.float32), np.asarray(bhh1, np.float32),
        np.asarray(W_dec, np.float32), np.asarray(b_dec, np.float32))

    nc = _get_program(_n_tf, _n_ar)
    in_maps = _make_in_maps(x, weights, _n_tf)
    res = run_bass_kernel_spmd(nc, in_maps, core_ids=list(range(NCORES)))

    out = np.empty((B, _n_ar + 1, F), np.float32)
    for c in range(NCORES):
        y = np.asarray(res.results[c]["y"], dtype=np.float32)  # [n_out, F, BL]
        out[c * BL:(c + 1) * BL] = y.transpose(2, 0, 1)
    return out
